# revision 41
# baseline (speedup 1.0000x reference)
"""HardMaxAttention Trainium2 Bass kernel (v2: fp16 hi/lo QK + K=6 scores).

Reference computation (per batch b):
    Q = x @ W_Q.T            (T, 2)
    K = x @ W_K.T            (T, 2)
    scores = Q @ K.T         (T, T), causal-masked (strict upper tri = -inf)
    idx = argmax(scores, -1) (T,)
    out = x[idx] @ W_V.T     (T, D)   [== take_along_axis(V, idx)]

Sharding: 8 cores = 4 batches x 2 t-parity shards (as v1).  Core c gets
batch b=c//2, parity h=c%2; x[b] rows are permuted so own tiles occupy
positions 0..2047, other parity 2048..4095.

Precision scheme (the argmax is intolerant of low-precision scores --
bf16 flips ~90 rows, fp32r ~11; fp32 matmuls cost 4 cycles/row):
  - x and W_Q/W_K are split hi/lo into fp16 on the host (x = xh + xl
    exactly to ~2^-24).  Q^T/K^T accumulate in PSUM fp32 from 3 fp16
    matmul terms (Wh xh + Wh xl + Wl xh); error ~2^-24.
  - The PE computes q rows triplicated (M=6, lhsT cols [W,W,W]) so the
    stacked hi/lo score operands can be extracted with partition-aligned
    casts/subs only: qs = [qh qh ql] (rows 0:6), ks = [kh kl kh] (rows
    32:38 via tile_position col group 1).
  - scores = qh.kh + qh.kl + ql.kh as ONE K=6 fp16 matmul per chunk
    (1 cycle/row); error ~2^-22 -> no argmax flips.
  - V path in bf16; output stored bf16 and upcast on host.
"""

import numpy as np

B, T, D, H = 4, 4096, 1024, 2
P = 128
NT = T // P            # 32 t-tiles per batch
MYT = NT // 2          # 16 t-tiles per core
KD = D // P            # 8 contraction blocks
NG = T // 512          # 8 QK groups (4 own-parity, 4 other-parity)
N_CORES = 8
NEG = -1.0e30

_prog_cache = {}


def _build_program():
    import concourse.bacc as bacc
    import concourse.mybir as mybir
    import concourse.tile as tile
    import concourse.bass as bass
    from concourse.masks import make_identity

    f32 = mybir.dt.float32
    f16 = mybir.dt.float16
    bf16 = mybir.dt.bfloat16
    u32 = mybir.dt.uint32

    nc = bacc.Bacc(None, target_bir_lowering=False)

    # x^T in group layout, fp16 hi/lo: xq*[g, p, k*512+c] = x_perm[g*512+c, k*128+p]
    xqh = nc.dram_tensor("xqh", [NG, P, KD * 512], f16, kind="ExternalInput")
    xql = nc.dram_tensor("xql", [NG, P, KD * 512], f16, kind="ExternalInput")
    # gather + V-projection source
    xv = nc.dram_tensor("xv", [T, D], bf16, kind="ExternalInput")
    # weights [D, 12]: cols = [Wq Wq Wq Wk Wk Wk] (2 cols each), hi/lo fp16
    w12h = nc.dram_tensor("w12h", [D, 12], f16, kind="ExternalInput")
    w12l = nc.dram_tensor("w12l", [D, 12], f16, kind="ExternalInput")
    w_vT = nc.dram_tensor("w_vT", [D, D], bf16, kind="ExternalInput")
    dmask = nc.dram_tensor("dmask", [P, P], f32, kind="ExternalInput")
    tmask = nc.dram_tensor("tmask", [P, P], f32, kind="ExternalInput")
    out = nc.dram_tensor("out", [MYT, P, D], bf16, kind="ExternalOutput")

    # group emission order: own-parity g alternating with other-parity g+4,
    # so tiles 4j..4j+3 unlock after pair (j, j+4).
    def gpair(j):
        return (j, j + 4)

    with tile.TileContext(nc) as tc:
        with (
            tc.tile_pool(name="const", bufs=1) as cpool,
            tc.tile_pool(name="xin", bufs=3) as xpool,
            tc.tile_pool(name="xt", bufs=3) as xtpool,
            tc.tile_pool(name="qk", bufs=1) as qkpool,
            tc.tile_pool(name="sc", bufs=4) as scpool,
            tc.tile_pool(name="small", bufs=4) as spool,
            tc.tile_pool(name="xg", bufs=3) as xgpool,
            tc.tile_pool(name="ob", bufs=3) as opool,
            tc.tile_pool(name="tp_ps", bufs=2, space="PSUM") as tpsum,
            tc.tile_pool(name="mm_ps", bufs=4, space="PSUM") as mmpsum,
            tc.tile_pool(name="vo_ps", bufs=2, space="PSUM") as vopsum,
        ):
            # ---- constants ----
            ident = cpool.tile([P, P], bf16)
            make_identity(nc, ident[:])
            # small/constant DMAs go on the scalar queue so the sync (SP)
            # queue starts the big xq loads immediately
            wh_sb = cpool.tile([P, 12 * KD], f16)
            wl_sb = cpool.tile([P, 12 * KD], f16)
            for k in range(KD):
                nc.gpsimd.dma_start(wh_sb[:, k * 12:(k + 1) * 12],
                                    w12h[k * P:(k + 1) * P, :])
                nc.gpsimd.dma_start(wl_sb[:, k * 12:(k + 1) * 12],
                                    w12l[k * P:(k + 1) * P, :])
            dmask_sb = cpool.tile([P, P], f32)
            nc.gpsimd.dma_start(dmask_sb[:], dmask[:])
            tmask_sb = cpool.tile([P, P], f32)
            nc.gpsimd.dma_start(tmask_sb[:], tmask[:])

            # stacked hi/lo score operands (both base partition 0), paired
            # rows contract together: qs6 = [ql qh qh], ks6 = [kh kl kh]
            # -> ql.kh + qh.kl + qh.kh
            qs6 = qkpool.tile([6, T], f16, tag="qs6")
            ks6 = qkpool.tile([6, T], f16, tag="ks6")

            wv_sb = cpool.tile([P, KD * D], bf16)

            # warm the PE (HAM un-throttle) during the initial xq DMA wait:
            # ~5us of dummy matmuls on the already-loaded weight tiles
            wps = mmpsum.tile([P, 512], f32, space="PSUM", tag="mmps")
            for wi in range(48):
                nc.tensor.matmul(
                    wps[0:12, 0:96],
                    lhsT=wh_sb[:, 0:12], rhs=wl_sb[:, 0:96],
                    start=True, stop=True,
                )

            def emit_group(g):
                """QK projection for 512 positions [g*512, (g+1)*512)."""
                xh_sb = xpool.tile([P, KD * 512], f16, tag="xh")
                xl_sb = xpool.tile([P, KD * 512], f16, tag="xl")
                nc.sync.dma_start(xh_sb[:], xqh[g, :, :])
                nc.scalar.dma_start(xl_sb[:], xql[g, :, :])
                # single M=12 matmul per hi/lo term per d-chunk:
                # psum rows 0:12 = [q q q k k k] (pairs), fp32 accumulate
                ps = mmpsum.tile([P, 512], f32, space="PSUM", tag="mmps")
                terms = ((wh_sb, xh_sb), (wh_sb, xl_sb), (wl_sb, xh_sb))
                n = len(terms) * KD
                i = 0
                for (w, xs) in terms:
                    for k in range(KD):
                        nc.tensor.matmul(
                            ps[0:12, :],
                            lhsT=w[:, k * 12:(k + 1) * 12],
                            rhs=xs[:, k * 512:(k + 1) * 512],
                            start=(i == 0), stop=(i == n - 1),
                        )
                        i += 1
                c0, c1 = g * 512, (g + 1) * 512
                # stage hi (fp16 cast) and lo (fp32 - hi) for all 12 rows
                # with base-0 ops, then assemble the stacked operands:
                # qs6 = [ql qh qh], ks6 = [kh kl kh]
                hi12 = spool.tile([12, 512], f16, tag="hi12")
                lo12 = spool.tile([12, 512], f16, tag="lo12")
                nc.scalar.copy(hi12[0:12, :], ps[0:12, :])
                nc.vector.tensor_tensor(
                    out=lo12[0:12, :], in0=ps[0:12, :], in1=hi12[0:12, :],
                    op=mybir.AluOpType.subtract,
                )
                nc.gpsimd.tensor_copy(qs6[0:2, c0:c1], lo12[0:2, :])  # ql
                nc.sync.dma_start(qs6[2:6, c0:c1], hi12[2:6, :])      # qh qh
                nc.scalar.dma_start(ks6[0:2, c0:c1], hi12[6:8, :])    # kh
                nc.sync.dma_start(ks6[2:4, c0:c1], lo12[6:8, :])      # kl
                nc.scalar.dma_start(ks6[4:6, c0:c1], hi12[8:10, :])   # kh

            xg_tiles = {}

            def emit_scores(i, cp=[0]):
                E = (i + 1) * P
                W = 2 * E
                sc = scpool.tile([P, 2 * MYT * P], f32)

                def chunk_copy(dst, src):
                    # PSUM->SBUF drain: ACT mostly, DVE for every 4th chunk
                    # (only ACT/DVE can read PSUM)
                    if cp[0] % 5 == 4:
                        nc.vector.tensor_copy(dst, src)
                    else:
                        nc.scalar.copy(dst, src)
                    cp[0] += 1

                for (base_src, base_dst, mk) in (
                    (0, 0, dmask_sb),
                    (T // 2, E, tmask_sb),
                ):
                    for c0 in range(0, E, 512):
                        c1 = min(E, c0 + 512)
                        nn = c1 - c0
                        ps = mmpsum.tile([P, 512], f32, space="PSUM",
                                         tag="mmps")
                        nc.tensor.matmul(
                            ps[0:P, :nn],
                            lhsT=qs6[0:6, i * P:(i + 1) * P],
                            rhs=ks6[0:6, base_src + c0:base_src + c1],
                            start=True, stop=True,
                        )
                        if c1 == E:
                            if nn > P:
                                chunk_copy(
                                    sc[:, base_dst + c0:base_dst + c1 - P],
                                    ps[0:P, :nn - P],
                                )
                            nc.vector.tensor_tensor(
                                out=sc[:, base_dst + E - P:base_dst + E],
                                in0=ps[0:P, nn - P:nn],
                                in1=mk[:],
                                op=mybir.AluOpType.add,
                            )
                        else:
                            chunk_copy(
                                sc[:, base_dst + c0:base_dst + c1],
                                ps[0:P, :nn],
                            )

                mx8 = spool.tile([P, 8], f32, tag="mx8")
                ix8 = spool.tile([P, 8], u32, tag="ix8")
                nc.vector.max(out=mx8[:], in_=sc[:, :W])
                nc.vector.max_index(out=ix8[:], in_max=mx8[:],
                                    in_values=sc[:, :W])

                # positions >= E belong to range B: add (2048 - E)
                idxf = spool.tile([P, 1], f32, tag="idxf")
                gef = spool.tile([P, 1], f32, tag="gef")
                idxu = spool.tile([P, 1], u32, tag="idxu")
                nc.gpsimd.tensor_copy(idxf[:], ix8[:, 0:1])
                nc.gpsimd.tensor_scalar(
                    gef[:], idxf[:], float(E), scalar2=None,
                    op0=mybir.AluOpType.is_ge,
                )
                nc.gpsimd.tensor_scalar(
                    gef[:], gef[:], float(T // 2 - E), scalar2=None,
                    op0=mybir.AluOpType.mult,
                )
                nc.gpsimd.tensor_tensor(
                    out=idxf[:], in0=idxf[:], in1=gef[:],
                    op=mybir.AluOpType.add,
                )
                nc.gpsimd.tensor_copy(idxu[:], idxf[:])

                xg = xgpool.tile([P, D], bf16)
                nc.gpsimd.indirect_dma_start(
                    out=xg[:],
                    out_offset=None,
                    in_=xv[:],
                    in_offset=bass.IndirectOffsetOnAxis(ap=idxu[:, 0:1],
                                                        axis=0),
                )
                xg_tiles[i] = xg

            def emit_vproj(i):
                xg = xg_tiles.pop(i)
                # 4 transposes share one PSUM tile -> 1 wide DVE copy per 4
                xgT = xtpool.tile([P, D], bf16, tag="xgt")
                for k4 in range(0, KD, 4):
                    tp = tpsum.tile([P, 512], bf16, space="PSUM", tag="tp")
                    for k in range(4):
                        nc.tensor.transpose(
                            tp[:, k * P:(k + 1) * P],
                            xg[:, (k4 + k) * P:(k4 + k + 1) * P], ident[:]
                        )
                    if k4 == 0:
                        nc.vector.tensor_copy(
                            xgT[:, k4 * P:(k4 + 4) * P], tp[:])
                    else:
                        nc.scalar.copy(xgT[:, k4 * P:(k4 + 4) * P], tp[:])

                ob = opool.tile([P, D], bf16)
                for n in range(2):
                    vo = vopsum.tile([P, 512], f32, space="PSUM", tag="vo")
                    for k in range(KD):
                        nc.tensor.matmul(
                            vo[:],
                            lhsT=xgT[:, k * P:(k + 1) * P],
                            rhs=wv_sb[:, k * D + n * 512:k * D + n * 512 + 512],
                            start=(k == 0),
                            stop=(k == KD - 1),
                        )
                    nc.scalar.copy(ob[:, n * 512:(n + 1) * 512], vo[:])
                nc.sync.dma_start(out[i, :, :], ob[:])

            # software pipeline: scores(i) runs LAG tiles ahead of the
            # transpose+Vproj tail so the PE never waits on a gather.
            # Tiles 0-2 run LAST: their tiny argmax chains shrink the tail.
            LAG = 3
            order = [0, 1, 2, 3, 4, 5, 6, 7, 8, 9, 10, 11, 12, 13, 14, 15]
            step = 0
            for j in range(4):
                emit_group(j)
                emit_group(j + 4)
                if j == 0:
                    # W_V load after first group pair's DMAs are queued
                    for k in range(KD):
                        nc.gpsimd.dma_start(
                            wv_sb[:, k * D:(k + 1) * D],
                            w_vT[k * P:(k + 1) * P, :],
                        )
                nt = 4
                for _ in range(nt):
                    if step - LAG >= 0:
                        emit_vproj(order[step - LAG])
                    emit_scores(order[step])
                    step += 1
            for s in range(MYT - LAG, MYT):
                emit_vproj(order[s])

    nc.compile()
    return nc


def get_program():
    if "nc" not in _prog_cache:
        _prog_cache["nc"] = _build_program()
    return _prog_cache["nc"]


def _hilo(a):
    """Exact fp16 hi/lo split: a == hi + lo to ~2^-24."""
    hi = a.astype(np.float16)
    lo = (a - hi.astype(np.float32)).astype(np.float16)
    return hi, lo


def make_core_inputs(x_full, W_Q, W_K, W_V):
    import ml_dtypes

    x_full = np.ascontiguousarray(x_full, dtype=np.float32)
    W_Q = np.asarray(W_Q, np.float32)
    W_K = np.asarray(W_K, np.float32)
    w_vT = np.ascontiguousarray(
        np.asarray(W_V, np.float32).T.astype(ml_dtypes.bfloat16))

    # [D, 12] = [Wq.T x3 | Wk.T x3], split hi/lo fp16
    w12 = np.concatenate([W_Q.T] * 3 + [W_K.T] * 3, axis=1)  # (D, 12)
    w12h, w12l = _hilo(w12)
    w12h = np.ascontiguousarray(w12h)
    w12l = np.ascontiguousarray(w12l)

    r = np.arange(P)
    dmask = np.where(r[None, :] <= r[:, None], 0.0, NEG).astype(np.float32)

    in_maps = []
    tiles_per_core = []
    for c in range(N_CORES):
        b, h = divmod(c, 2)
        mine = [2 * i + h for i in range(MYT)]
        other = [2 * i + (1 - h) for i in range(MYT)]
        rows = np.concatenate(
            [np.arange(t * P, (t + 1) * P) for t in mine + other]
        )
        xb_perm = np.ascontiguousarray(x_full[b][rows])
        xh, xl = _hilo(xb_perm)
        # transposed group layout [NG, P, KD*512]
        def gl(a):
            return np.ascontiguousarray(
                a.reshape(NG, 512, KD, P).transpose(0, 3, 2, 1)
                .reshape(NG, P, KD * 512))
        tmask = np.full((P, P), NEG if h == 0 else 0.0, dtype=np.float32)
        in_maps.append({
            "xqh": gl(xh), "xql": gl(xl),
            "xv": np.ascontiguousarray(xb_perm.astype(ml_dtypes.bfloat16)),
            "w12h": w12h, "w12l": w12l,
            "w_vT": w_vT, "dmask": dmask, "tmask": tmask,
        })
        tiles_per_core.append(mine)
    return in_maps, tiles_per_core


def assemble_output(results, tiles_per_core):
    out_full = np.empty((B, T, D), dtype=np.float32)
    for c in range(N_CORES):
        b = c // 2
        oc = np.asarray(results[c]["out"], dtype=np.float32)
        for i, th in enumerate(tiles_per_core[c]):
            out_full[b, th * P:(th + 1) * P, :] = oc[i]
    return out_full


def kernel(**inputs):
    from concourse.bass_utils import run_bass_kernel_spmd

    x_full = np.asarray(inputs["x"], dtype=np.float32)
    in_maps, tiles_per_core = make_core_inputs(
        x_full, np.asarray(inputs["W_Q"]), np.asarray(inputs["W_K"]),
        np.asarray(inputs["W_V"])
    )
    nc = get_program()
    res = run_bass_kernel_spmd(nc, in_maps, core_ids=list(range(N_CORES)))
    return assemble_output(res.results, tiles_per_core)


# revision 42
# speedup vs baseline: 1.0514x; 1.0514x over previous
"""HardMaxAttention Trainium2 Bass kernel (v2: fp16 hi/lo QK + K=6 scores).

Reference computation (per batch b):
    Q = x @ W_Q.T            (T, 2)
    K = x @ W_K.T            (T, 2)
    scores = Q @ K.T         (T, T), causal-masked (strict upper tri = -inf)
    idx = argmax(scores, -1) (T,)
    out = x[idx] @ W_V.T     (T, D)   [== take_along_axis(V, idx)]

Sharding: 8 cores = 4 batches x 2 t-parity shards (as v1).  Core c gets
batch b=c//2, parity h=c%2; x[b] rows are permuted so own tiles occupy
positions 0..2047, other parity 2048..4095.

Precision scheme (the argmax is intolerant of low-precision scores --
bf16 flips ~90 rows, fp32r ~11; fp32 matmuls cost 4 cycles/row):
  - x and W_Q/W_K are split hi/lo into fp16 on the host (x = xh + xl
    exactly to ~2^-24).  Q^T/K^T accumulate in PSUM fp32 from 3 fp16
    matmul terms (Wh xh + Wh xl + Wl xh); error ~2^-24.
  - The PE computes q rows triplicated (M=6, lhsT cols [W,W,W]) so the
    stacked hi/lo score operands can be extracted with partition-aligned
    casts/subs only: qs = [qh qh ql] (rows 0:6), ks = [kh kl kh] (rows
    32:38 via tile_position col group 1).
  - scores = qh.kh + qh.kl + ql.kh as ONE K=6 fp16 matmul per chunk
    (1 cycle/row); error ~2^-22 -> no argmax flips.
  - V path in bf16; output stored bf16 and upcast on host.
"""

import numpy as np

B, T, D, H = 4, 4096, 1024, 2
P = 128
NT = T // P            # 32 t-tiles per batch
MYT = NT // 2          # 16 t-tiles per core
KD = D // P            # 8 contraction blocks
NG = T // 512          # 8 QK groups (4 own-parity, 4 other-parity)
N_CORES = 8
NEG = -1.0e30

_prog_cache = {}


def _build_program():
    import concourse.bacc as bacc
    import concourse.mybir as mybir
    import concourse.tile as tile
    import concourse.bass as bass
    from concourse.masks import make_identity

    f32 = mybir.dt.float32
    f16 = mybir.dt.float16
    bf16 = mybir.dt.bfloat16
    u32 = mybir.dt.uint32

    nc = bacc.Bacc(None, target_bir_lowering=False)

    # x^T in group layout, fp16 hi/lo: xq*[g, p, k*512+c] = x_perm[g*512+c, k*128+p]
    xqh = nc.dram_tensor("xqh", [NG, P, KD * 512], f16, kind="ExternalInput")
    xql = nc.dram_tensor("xql", [NG, P, KD * 512], f16, kind="ExternalInput")
    # gather + V-projection source
    xv = nc.dram_tensor("xv", [T, D], bf16, kind="ExternalInput")
    # weights [D, 12]: cols = [Wq Wq Wq Wk Wk Wk] (2 cols each), hi/lo fp16
    w12h = nc.dram_tensor("w12h", [D, 12], f16, kind="ExternalInput")
    w12l = nc.dram_tensor("w12l", [D, 12], f16, kind="ExternalInput")
    w_vT = nc.dram_tensor("w_vT", [D, D], bf16, kind="ExternalInput")
    dmask = nc.dram_tensor("dmask", [P, P], f32, kind="ExternalInput")
    tmask = nc.dram_tensor("tmask", [P, P], f32, kind="ExternalInput")
    out = nc.dram_tensor("out", [MYT, P, D], bf16, kind="ExternalOutput")

    # group emission order: own-parity g alternating with other-parity g+4,
    # so tiles 4j..4j+3 unlock after pair (j, j+4).
    def gpair(j):
        return (j, j + 4)

    with tile.TileContext(nc) as tc:
        with (
            tc.tile_pool(name="const", bufs=1) as cpool,
            tc.tile_pool(name="xin", bufs=3) as xpool,
            tc.tile_pool(name="xt", bufs=3) as xtpool,
            tc.tile_pool(name="qk", bufs=1) as qkpool,
            tc.tile_pool(name="sc", bufs=5) as scpool,
            tc.tile_pool(name="small", bufs=6) as spool,
            tc.tile_pool(name="xg", bufs=5) as xgpool,
            tc.tile_pool(name="ob", bufs=3) as opool,
            tc.tile_pool(name="tp_ps", bufs=2, space="PSUM") as tpsum,
            tc.tile_pool(name="mm_ps", bufs=4, space="PSUM") as mmpsum,
            tc.tile_pool(name="vo_ps", bufs=2, space="PSUM") as vopsum,
        ):
            # ---- constants ----
            ident = cpool.tile([P, P], bf16)
            make_identity(nc, ident[:])
            # small/constant DMAs go on the scalar queue so the sync (SP)
            # queue starts the big xq loads immediately
            wh_sb = cpool.tile([P, 12 * KD], f16)
            wl_sb = cpool.tile([P, 12 * KD], f16)
            for k in range(KD):
                nc.gpsimd.dma_start(wh_sb[:, k * 12:(k + 1) * 12],
                                    w12h[k * P:(k + 1) * P, :])
                nc.gpsimd.dma_start(wl_sb[:, k * 12:(k + 1) * 12],
                                    w12l[k * P:(k + 1) * P, :])
            dmask_sb = cpool.tile([P, P], f32)
            nc.gpsimd.dma_start(dmask_sb[:], dmask[:])
            tmask_sb = cpool.tile([P, P], f32)
            nc.gpsimd.dma_start(tmask_sb[:], tmask[:])

            # stacked hi/lo score operands (both base partition 0), paired
            # rows contract together: qs6 = [ql qh qh], ks6 = [kh kl kh]
            # -> ql.kh + qh.kl + qh.kh
            qs6 = qkpool.tile([6, T], f16, tag="qs6")
            ks6 = qkpool.tile([6, T], f16, tag="ks6")

            wv_sb = cpool.tile([P, KD * D], bf16)

            # warm the PE (HAM un-throttle) during the initial xq DMA wait:
            # ~5us of dummy matmuls on the already-loaded weight tiles
            wps = mmpsum.tile([P, 512], f32, space="PSUM", tag="mmps")
            for wi in range(24):
                nc.tensor.matmul(
                    wps[0:12, 0:96],
                    lhsT=wh_sb[:, 0:12], rhs=wl_sb[:, 0:96],
                    start=True, stop=True,
                )

            def emit_group(g):
                """QK projection for 512 positions [g*512, (g+1)*512)."""
                xh_sb = xpool.tile([P, KD * 512], f16, tag="xh")
                xl_sb = xpool.tile([P, KD * 512], f16, tag="xl")
                nc.sync.dma_start(xh_sb[:], xqh[g, :, :])
                nc.scalar.dma_start(xl_sb[:], xql[g, :, :])
                # single M=12 matmul per hi/lo term per d-chunk:
                # psum rows 0:12 = [q q q k k k] (pairs), fp32 accumulate
                ps = mmpsum.tile([P, 512], f32, space="PSUM", tag="mmps")
                terms = ((wh_sb, xh_sb), (wh_sb, xl_sb), (wl_sb, xh_sb))
                n = len(terms) * KD
                i = 0
                for (w, xs) in terms:
                    for k in range(KD):
                        nc.tensor.matmul(
                            ps[0:12, :],
                            lhsT=w[:, k * 12:(k + 1) * 12],
                            rhs=xs[:, k * 512:(k + 1) * 512],
                            start=(i == 0), stop=(i == n - 1),
                        )
                        i += 1
                c0, c1 = g * 512, (g + 1) * 512
                # stage hi (fp16 cast) and lo (fp32 - hi) for all 12 rows
                # with base-0 ops, then assemble the stacked operands:
                # qs6 = [ql qh qh], ks6 = [kh kl kh]
                hi12 = spool.tile([12, 512], f16, tag="hi12")
                lo12 = spool.tile([12, 512], f16, tag="lo12")
                nc.scalar.copy(hi12[0:12, :], ps[0:12, :])
                nc.vector.tensor_tensor(
                    out=lo12[0:12, :], in0=ps[0:12, :], in1=hi12[0:12, :],
                    op=mybir.AluOpType.subtract,
                )
                nc.gpsimd.tensor_copy(qs6[0:2, c0:c1], lo12[0:2, :])  # ql
                nc.sync.dma_start(qs6[2:6, c0:c1], hi12[2:6, :])      # qh qh
                nc.scalar.dma_start(ks6[0:2, c0:c1], hi12[6:8, :])    # kh
                nc.sync.dma_start(ks6[2:4, c0:c1], lo12[6:8, :])      # kl
                nc.scalar.dma_start(ks6[4:6, c0:c1], hi12[8:10, :])   # kh

            xg_tiles = {}

            def emit_scores(i, cp=[0]):
                E = (i + 1) * P
                W = 2 * E
                sc = scpool.tile([P, 2 * MYT * P], f32)

                def chunk_copy(dst, src):
                    # PSUM->SBUF drain: ACT mostly, DVE for every 4th chunk
                    # (only ACT/DVE can read PSUM)
                    if cp[0] % 8 == 7:
                        nc.vector.tensor_copy(dst, src)
                    else:
                        nc.scalar.copy(dst, src)
                    cp[0] += 1

                for (base_src, base_dst, mk) in (
                    (0, 0, dmask_sb),
                    (T // 2, E, tmask_sb),
                ):
                    for c0 in range(0, E, 512):
                        c1 = min(E, c0 + 512)
                        nn = c1 - c0
                        ps = mmpsum.tile([P, 512], f32, space="PSUM",
                                         tag="mmps")
                        nc.tensor.matmul(
                            ps[0:P, :nn],
                            lhsT=qs6[0:6, i * P:(i + 1) * P],
                            rhs=ks6[0:6, base_src + c0:base_src + c1],
                            start=True, stop=True,
                        )
                        if c1 == E:
                            if nn > P:
                                chunk_copy(
                                    sc[:, base_dst + c0:base_dst + c1 - P],
                                    ps[0:P, :nn - P],
                                )
                            nc.vector.tensor_tensor(
                                out=sc[:, base_dst + E - P:base_dst + E],
                                in0=ps[0:P, nn - P:nn],
                                in1=mk[:],
                                op=mybir.AluOpType.add,
                            )
                        else:
                            chunk_copy(
                                sc[:, base_dst + c0:base_dst + c1],
                                ps[0:P, :nn],
                            )

                mx8 = spool.tile([P, 8], f32, tag="mx8")
                ix8 = spool.tile([P, 8], u32, tag="ix8")
                nc.vector.max(out=mx8[:], in_=sc[:, :W])
                nc.vector.max_index(out=ix8[:], in_max=mx8[:],
                                    in_values=sc[:, :W])

                # positions >= E belong to range B: add (2048 - E)
                idxf = spool.tile([P, 1], f32, tag="idxf")
                gef = spool.tile([P, 1], f32, tag="gef")
                idxu = spool.tile([P, 1], u32, tag="idxu")
                nc.gpsimd.tensor_copy(idxf[:], ix8[:, 0:1])
                nc.gpsimd.tensor_scalar(
                    gef[:], idxf[:], float(E), scalar2=None,
                    op0=mybir.AluOpType.is_ge,
                )
                nc.gpsimd.tensor_scalar(
                    gef[:], gef[:], float(T // 2 - E), scalar2=None,
                    op0=mybir.AluOpType.mult,
                )
                nc.gpsimd.tensor_tensor(
                    out=idxf[:], in0=idxf[:], in1=gef[:],
                    op=mybir.AluOpType.add,
                )
                nc.gpsimd.tensor_copy(idxu[:], idxf[:])

                xg = xgpool.tile([P, D], bf16)
                nc.gpsimd.indirect_dma_start(
                    out=xg[:],
                    out_offset=None,
                    in_=xv[:],
                    in_offset=bass.IndirectOffsetOnAxis(ap=idxu[:, 0:1],
                                                        axis=0),
                )
                xg_tiles[i] = xg

            def emit_vproj(i):
                xg = xg_tiles.pop(i)
                # 4 transposes share one PSUM tile -> 1 wide DVE copy per 4
                xgT = xtpool.tile([P, D], bf16, tag="xgt")
                for k4 in range(0, KD, 4):
                    tp = tpsum.tile([P, 512], bf16, space="PSUM", tag="tp")
                    for k in range(4):
                        nc.tensor.transpose(
                            tp[:, k * P:(k + 1) * P],
                            xg[:, (k4 + k) * P:(k4 + k + 1) * P], ident[:]
                        )
                    if k4 == 0:
                        nc.vector.tensor_copy(
                            xgT[:, k4 * P:(k4 + 4) * P], tp[:])
                    else:
                        nc.scalar.copy(xgT[:, k4 * P:(k4 + 4) * P], tp[:])

                ob = opool.tile([P, D], bf16)
                for n in range(2):
                    vo = vopsum.tile([P, 512], f32, space="PSUM", tag="vo")
                    for k in range(KD):
                        nc.tensor.matmul(
                            vo[:],
                            lhsT=xgT[:, k * P:(k + 1) * P],
                            rhs=wv_sb[:, k * D + n * 512:k * D + n * 512 + 512],
                            start=(k == 0),
                            stop=(k == KD - 1),
                        )
                    nc.scalar.copy(ob[:, n * 512:(n + 1) * 512], vo[:])
                nc.sync.dma_start(out[i, :, :], ob[:])

            # software pipeline: scores(i) runs LAG tiles ahead of the
            # transpose+Vproj tail so the PE never waits on a gather.
            # Tiles 0-2 run LAST: their tiny argmax chains shrink the tail.
            LAG = 4
            order = [0, 1, 2, 3, 4, 5, 6, 7, 8, 9, 10, 11, 12, 13, 14, 15]
            step = 0
            for j in range(4):
                emit_group(j)
                emit_group(j + 4)
                if j == 0:
                    # W_V load after first group pair's DMAs are queued
                    for k in range(KD):
                        nc.gpsimd.dma_start(
                            wv_sb[:, k * D:(k + 1) * D],
                            w_vT[k * P:(k + 1) * P, :],
                        )
                nt = 4
                for _ in range(nt):
                    if step - LAG >= 0:
                        emit_vproj(order[step - LAG])
                    emit_scores(order[step])
                    step += 1
            for s in range(MYT - LAG, MYT):
                emit_vproj(order[s])

    nc.compile()
    return nc


def get_program():
    if "nc" not in _prog_cache:
        _prog_cache["nc"] = _build_program()
    return _prog_cache["nc"]


def _hilo(a):
    """Exact fp16 hi/lo split: a == hi + lo to ~2^-24."""
    hi = a.astype(np.float16)
    lo = (a - hi.astype(np.float32)).astype(np.float16)
    return hi, lo


def make_core_inputs(x_full, W_Q, W_K, W_V):
    import ml_dtypes

    x_full = np.ascontiguousarray(x_full, dtype=np.float32)
    W_Q = np.asarray(W_Q, np.float32)
    W_K = np.asarray(W_K, np.float32)
    w_vT = np.ascontiguousarray(
        np.asarray(W_V, np.float32).T.astype(ml_dtypes.bfloat16))

    # [D, 12] = [Wq.T x3 | Wk.T x3], split hi/lo fp16
    w12 = np.concatenate([W_Q.T] * 3 + [W_K.T] * 3, axis=1)  # (D, 12)
    w12h, w12l = _hilo(w12)
    w12h = np.ascontiguousarray(w12h)
    w12l = np.ascontiguousarray(w12l)

    r = np.arange(P)
    dmask = np.where(r[None, :] <= r[:, None], 0.0, NEG).astype(np.float32)

    in_maps = []
    tiles_per_core = []
    for c in range(N_CORES):
        b, h = divmod(c, 2)
        mine = [2 * i + h for i in range(MYT)]
        other = [2 * i + (1 - h) for i in range(MYT)]
        rows = np.concatenate(
            [np.arange(t * P, (t + 1) * P) for t in mine + other]
        )
        xb_perm = np.ascontiguousarray(x_full[b][rows])
        xh, xl = _hilo(xb_perm)
        # transposed group layout [NG, P, KD*512]
        def gl(a):
            return np.ascontiguousarray(
                a.reshape(NG, 512, KD, P).transpose(0, 3, 2, 1)
                .reshape(NG, P, KD * 512))
        tmask = np.full((P, P), NEG if h == 0 else 0.0, dtype=np.float32)
        in_maps.append({
            "xqh": gl(xh), "xql": gl(xl),
            "xv": np.ascontiguousarray(xb_perm.astype(ml_dtypes.bfloat16)),
            "w12h": w12h, "w12l": w12l,
            "w_vT": w_vT, "dmask": dmask, "tmask": tmask,
        })
        tiles_per_core.append(mine)
    return in_maps, tiles_per_core


def assemble_output(results, tiles_per_core):
    out_full = np.empty((B, T, D), dtype=np.float32)
    for c in range(N_CORES):
        b = c // 2
        oc = np.asarray(results[c]["out"], dtype=np.float32)
        for i, th in enumerate(tiles_per_core[c]):
            out_full[b, th * P:(th + 1) * P, :] = oc[i]
    return out_full


def kernel(**inputs):
    from concourse.bass_utils import run_bass_kernel_spmd

    x_full = np.asarray(inputs["x"], dtype=np.float32)
    in_maps, tiles_per_core = make_core_inputs(
        x_full, np.asarray(inputs["W_Q"]), np.asarray(inputs["W_K"]),
        np.asarray(inputs["W_V"])
    )
    nc = get_program()
    res = run_bass_kernel_spmd(nc, in_maps, core_ids=list(range(N_CORES)))
    return assemble_output(res.results, tiles_per_core)
